# revision 37
# baseline (speedup 1.0000x reference)
"""Bipartite 2-layer SAGEConv GNN on 8 Trainium2 NeuronCores.

Strategy (v2):
  - Edges sharded by destination range (core c owns dst rows [S*c, S*(c+1))
    for BOTH directions). Per core+direction, dsts sorted by degree; edges
    packed into 2-slot segments, 64 dst-rows per PSUM block, variable tiles
    per block (schedule = max over cores -> one SPMD program for all cores).
  - All tables/messages/weights fp16 (PE 1 cycle/row vs 4 for fp32).
  - Segment-sum TRANSPOSED: out[feat, dst] = msg_tile^T @ R8inv where
    R8inv is the one-hot segment matrix pre-scaled by 1/deg on the host.
    Output free size = 64 (dsts) and arrives pre-transposed for the GEMMs,
    so the mean pass and all input-side transposes disappear.
  - Message gather: dma_gather from fp16 tables (256B rows), CENTERED base
    (idx int16, idx = row - N/2). Gather calls have variable tile counts:
    call boundaries are chosen so every call's last index can be made
    non-negative by an order-only swap within its dst row (the SWDGE ucode
    trims trailing negatives). No dst-permutation mutation needed.
  - Layer 2 transform-first: z = x1 @ w2l.T (64 wide) is stored in a
    combined fp16 table zc[i] = [zu | zp] (256B rows) in DEGREE-PERMUTED
    order (host remaps layer-2 gather indices), so the z store is a plain
    strided DMA. One AllGather of zc.
  - lin_r terms (r2) and layer-2 means are emitted in transposed, permuted
    form; the host un-permutes/adds/biases them for free in numpy.
  - No dma_scatter_add anywhere.
"""
import sys
import numpy as np

sys.path.insert(0, "/opt/trn_rl_repo")

# ---------------- problem dims (hardcoded for the harness) ----------------
N = 50000
E = 800000
F_IN = 128
HID = 256
CLS = 64
NCORES = 8

SEG = 2            # slots per segment (one dst's edges per tile-row)
BPD = 64           # dsts per psum block
CHUNK_TILES = 8    # max tiles per gather call (1024 idx = HW SWDGE limit)
GRP = 512          # dst columns per phase-3 GEMM group


class CFG:
    def __init__(self, n=N, e=E, center=None):
        self.N = n
        self.E = e
        self.S = n // NCORES          # dst rows per core
        self.CENTER = n // 2 if center is None else center
        self.ZROW = n                 # zero row index of gather tables
        self.NB = -(-self.S // BPD)   # blocks per direction
        self.RT = -(-self.S // 128)   # 128-row tiles of the slice
        self.SP = self.NB * BPD       # padded rows


# ---------------- host-side edge scheduling ----------------

def _prep_dir(src_g, dst_g, c, cfg):
    """Per-core, per-direction metadata. pi is frozen (pure degree sort)."""
    lo = c * cfg.S
    m = (dst_g >= lo) & (dst_g < lo + cfg.S)
    ls = src_g[m].astype(np.int64)
    ld = (dst_g[m] - lo).astype(np.int64)
    deg = np.bincount(ld, minlength=cfg.S)
    pi = np.argsort(-deg, kind="stable").astype(np.int64)
    order = np.argsort(ld, kind="stable")
    ls_s = ls[order]
    starts = np.zeros(cfg.S + 1, np.int64)
    starts[1:] = np.cumsum(deg)
    degp = np.zeros(cfg.NB * BPD, np.int64)
    degp[: cfg.S] = deg[pi]
    treq = np.maximum(
        1, -(-degp.reshape(cfg.NB, BPD).max(1) // SEG)
    ).astype(np.int64)
    return dict(pi=pi, deg=deg, starts=starts, ls_s=ls_s, degp=degp, treq=treq)


def _build_slots(meta, T, cfg):
    """Slot array [ntiles, 128] of src node ids (ZROW for dummies)."""
    pi, deg, starts, ls_s = meta["pi"], meta["deg"], meta["starts"], meta["ls_s"]
    total_tiles = int(T.sum())
    out = np.full((total_tiles, BPD, SEG), cfg.ZROW, np.int64)
    t0 = 0
    for b in range(cfg.NB):
        tb = int(T[b])
        blk = out[t0: t0 + tb]
        for mrow in range(BPD):
            r = BPD * b + mrow
            if r >= cfg.S:
                continue
            D = int(pi[r])
            d = int(deg[D])
            if d == 0:
                continue
            vals = np.full(tb * SEG, cfg.ZROW, np.int64)
            vals[:d] = ls_s[starts[D]: starts[D] + d]
            blk[:, mrow, :] = vals.reshape(tb, SEG)
        t0 += tb
    return out.reshape(total_tiles, 128)


def _tile_blocks(T):
    """block index per tile + block start tile."""
    nt = int(T.sum())
    row_of_tile = np.zeros(nt, np.int64)
    blk_start = np.zeros(len(T), np.int64)
    t0 = 0
    for b, tb in enumerate(T):
        blk_start[b] = t0
        row_of_tile[t0: t0 + int(tb)] = b
        t0 += int(tb)
    return row_of_tile, blk_start


def _fix_tails(arrs, T, cfg):
    """Choose shared gather-call boundaries and fix each per-core slot array
    so every call's final slot value is >= CENTER (order-only swaps within
    the dst row at partition 127).

    arrs: list (per core) of [nt, 128] slot-value arrays, mutated in place.
    Returns list of (t0, ct) gather calls.
    """
    nt = arrs[0].shape[0]
    row_of_tile, blk_start = _tile_blocks(T)
    reserved = [dict() for _ in arrs]   # core -> {block: set(stream pos)}

    def stream(a, b):
        """Flat slot stream of partition-127's dst row in block b."""
        tb = int(T[b])
        s0 = int(blk_start[b])
        return a[s0: s0 + tb, (BPD - 1) * SEG:].reshape(-1)

    def fix_one(a, res, b, jpos, apply):
        st = stream(a, b)
        r = res.setdefault(b, set())
        if st[jpos] >= cfg.CENTER:
            if apply:
                r.add(jpos)
            return True
        cand = np.nonzero(st >= cfg.CENTER)[0]
        cand = [j for j in cand if j not in r and j != jpos]
        if not cand:
            return False
        if apply:
            j = int(cand[0])
            tb = int(T[b])
            s0 = int(blk_start[b])
            view = a[s0: s0 + tb, (BPD - 1) * SEG:].reshape(-1)
            view[jpos], view[j] = view[j], view[jpos]
            a[s0: s0 + tb, (BPD - 1) * SEG:] = view.reshape(tb, SEG)
            r.add(jpos)
        return True

    calls = []
    t0 = 0
    while t0 < nt:
        ct = min(CHUNK_TILES, nt - t0)
        chosen = None
        for bnd in range(t0 + ct, t0, -1):
            tb1 = bnd - 1
            b = int(row_of_tile[tb1])
            jpos = int(tb1 - blk_start[b]) * SEG + (SEG - 1)
            if all(fix_one(arrs[c], reserved[c], b, jpos, False)
                   for c in range(len(arrs))):
                chosen = bnd
                for c in range(len(arrs)):
                    fix_one(arrs[c], reserved[c], b, jpos, True)
                break
        assert chosen is not None, "no fixable gather boundary in window"
        calls.append((t0, chosen - t0))
        t0 = chosen
    return calls


def _wrap16(idx16):
    """[n] int16 -> [128, n/16]: idx i at partition i%16, col i//16, x8."""
    n = len(idx16)
    assert n % 16 == 0
    return np.tile(idx16.reshape(n // 16, 16).T, (8, 1)).astype(np.int16)


def _prep_all(inputs, cfg):
    f16 = np.float16
    x_user = np.asarray(inputs["x_user"], np.float32)
    x_product = np.asarray(inputs["x_product"], np.float32)
    ei = np.asarray(inputs["edge_index"]).astype(np.int64)
    u, p = ei[0], ei[1]

    metaA = [_prep_dir(u, p, c, cfg) for c in range(NCORES)]  # dst=p, src=u
    metaB = [_prep_dir(p, u, c, cfg) for c in range(NCORES)]  # dst=u, src=p

    TA = np.max([m["treq"] for m in metaA], axis=0)
    TB = np.max([m["treq"] for m in metaB], axis=0)

    # layer-1 slot arrays + call schedules
    sl1A = [_build_slots(metaA[c], TA, cfg) for c in range(NCORES)]
    sl1B = [_build_slots(metaB[c], TB, cfg) for c in range(NCORES)]
    c1A = _fix_tails(sl1A, TA, cfg)
    c1B = _fix_tails(sl1B, TB, cfg)

    # z-table position maps (degree-permuted layout, global)
    PA = np.empty(cfg.N + 1, np.int64)
    PB = np.empty(cfg.N + 1, np.int64)
    ar = np.arange(cfg.S, dtype=np.int64)
    for c in range(NCORES):
        PA[c * cfg.S + metaA[c]["pi"]] = c * cfg.S + ar
        PB[c * cfg.S + metaB[c]["pi"]] = c * cfg.S + ar
    PA[cfg.N] = cfg.N
    PB[cfg.N] = cfg.N

    sl2A = [PA[a] for a in sl1A]
    sl2B = [PB[a] for a in sl1B]
    c2A = _fix_tails(sl2A, TA, cfg)
    c2B = _fix_tails(sl2B, TB, cfg)

    def tab(x):
        t = np.zeros((cfg.N + 1, F_IN), f16)
        t[: cfg.N] = x.astype(f16)
        return t

    w = {k: np.asarray(v, np.float32) for k, v in inputs.items()
         if k.startswith(("w_", "b_"))}

    r8 = np.repeat(np.eye(BPD, dtype=np.float32), SEG, axis=0)  # [128, BPD]

    def r8inv(meta):
        invc = 1.0 / np.maximum(meta["degp"], 1.0)               # [NB*BPD]
        m = r8[:, None, :] * invc.reshape(cfg.NB, BPD)[None, :, :]
        return np.ascontiguousarray(
            m.reshape(128, cfg.NB * BPD).astype(f16))

    def w2stack(wz, wr):
        # [128, 2, 128]: cols 0:64 = wz.T chunk, 64:128 = wr.T chunk
        s = np.zeros((128, 2, 128), f16)
        for h in range(2):
            s[:, h, 0:CLS] = wz.T[128 * h: 128 * (h + 1), :]
            s[:, h, CLS:128] = wr.T[128 * h: 128 * (h + 1), :]
        return np.ascontiguousarray(s.reshape(128, 256))

    shared = {
        "xu_tab": tab(x_user), "xp_tab": tab(x_product),
        "wu1l": np.ascontiguousarray(w["w_u1_l"].T.astype(f16)),
        "wu1r": np.ascontiguousarray(w["w_u1_r"].T.astype(f16)),
        "wp1l": np.ascontiguousarray(w["w_p1_l"].T.astype(f16)),
        "wp1r": np.ascontiguousarray(w["w_p1_r"].T.astype(f16)),
        "w2A": w2stack(w["w_u2_l"], w["w_p2_r"]),   # z=zu, r2 -> xp2
        "w2B": w2stack(w["w_p2_l"], w["w_u2_r"]),   # z=zp, r2 -> xu2
        "bu1": np.ascontiguousarray(w["b_u1"].reshape(2, 128).T.astype(np.float32)),
        "bp1": np.ascontiguousarray(w["b_p1"].reshape(2, 128).T.astype(np.float32)),
        "ident": np.eye(128, dtype=f16),
    }

    in_maps = []
    for c in range(NCORES):
        d = dict(shared)
        for tag, meta, sl1, sl2, xsrc in (
                ("A", metaA[c], sl1A[c], sl2A[c], x_product),
                ("B", metaB[c], sl1B[c], sl2B[c], x_user)):
            d[f"g1{tag}"] = _wrap16(
                (sl1.reshape(-1) - cfg.CENTER).astype(np.int16))
            d[f"g2{tag}"] = _wrap16(
                (sl2.reshape(-1) - cfg.CENTER).astype(np.int16))
            d[f"r8i{tag}"] = r8inv(meta)
            rows = c * cfg.S + meta["pi"]
            xdT = np.zeros((F_IN, cfg.SP), f16)
            xdT[:, : cfg.S] = xsrc[rows].T.astype(f16)
            d[f"xdT{tag}"] = np.ascontiguousarray(xdT)
        in_maps.append(d)

    host_ctx = {
        "piA": [metaA[c]["pi"] for c in range(NCORES)],
        "piB": [metaB[c]["pi"] for c in range(NCORES)],
        "b_u2": w["b_u2"], "b_p2": w["b_p2"],
    }
    return in_maps, (TA, c1A, c2A), (TB, c1B, c2B), host_ctx


# ---------------- device program ----------------

def _build_nc(cfg, schedA, schedB, local_mode=False):
    import concourse.bacc as bacc
    import concourse.mybir as mybir
    from concourse.tile import TileContext

    f32, f16, i16 = mybir.dt.float32, mybir.dt.float16, mybir.dt.int16
    AF = mybir.ActivationFunctionType

    nc = bacc.Bacc(None, target_bir_lowering=False, num_devices=NCORES,
                   dynamic_dma_scratch_size=49152, num_swdge_queues=1)

    S, SP, NB, CENTER = cfg.S, cfg.SP, cfg.NB, cfg.CENTER
    TA, c1A, c2A = schedA
    TB, c1B, c2B = schedB
    ntA, ntB = int(TA.sum()), int(TB.sum())
    GMAX = max(ntA, ntB) * 8

    # ---- DRAM declarations ----
    t_xu = nc.dram_tensor("xu_tab", [cfg.N + 1, F_IN], f16, kind="ExternalInput")
    t_xp = nc.dram_tensor("xp_tab", [cfg.N + 1, F_IN], f16, kind="ExternalInput")
    tw = {}
    for k in ["wu1l", "wu1r", "wp1l", "wp1r", "w2A", "w2B"]:
        tw[k] = nc.dram_tensor(k, [128, 256], f16, kind="ExternalInput")
    for k in ["bu1", "bp1"]:
        tw[k] = nc.dram_tensor(k, [128, 2], f32, kind="ExternalInput")
    t_ident = nc.dram_tensor("ident", [128, 128], f16, kind="ExternalInput")
    t_g = {}
    for k, nt in (("g1A", ntA), ("g1B", ntB), ("g2A", ntA), ("g2B", ntB)):
        t_g[k] = nc.dram_tensor(k, [128, nt * 8], i16, kind="ExternalInput")
    t_r8iA = nc.dram_tensor("r8iA", [128, SP], f16, kind="ExternalInput")
    t_r8iB = nc.dram_tensor("r8iB", [128, SP], f16, kind="ExternalInput")
    t_xdTA = nc.dram_tensor("xdTA", [F_IN, SP], f16, kind="ExternalInput")
    t_xdTB = nc.dram_tensor("xdTB", [F_IN, SP], f16, kind="ExternalInput")

    outs = {k: nc.dram_tensor(k, [CLS, SP], f16, kind="ExternalOutput")
            for k in ["mu", "ru", "mp", "rp"]}

    st_zu = nc.dram_tensor("zu_stage", [SP, CLS], f16)
    st_zp = nc.dram_tensor("zp_stage", [SP, CLS], f16)
    aspace = "Local" if local_mode else "Shared"
    # AllGather targets: tight contiguous rows (walrus requires contiguous
    # collective outputs)
    t_zul = nc.dram_tensor("zu_lin", [cfg.N, CLS], f16, addr_space=aspace)
    t_zpl = nc.dram_tensor("zp_lin", [cfg.N, CLS], f16, addr_space=aspace)
    # gather tables padded to 256B rows (gather granularity); cols 64:128
    # unused garbage, filled by a local strided expand copy after the cc
    t_zuf = nc.dram_tensor("zu_full", [cfg.N + 1, 128], f16)
    t_zpf = nc.dram_tensor("zp_full", [cfg.N + 1, 128], f16)

    with TileContext(nc) as tc:
        with tc.tile_pool(name="persist", bufs=1) as pp, \
             tc.tile_pool(name="gidx", bufs=2) as gp, \
             tc.tile_pool(name="big", bufs=3) as bigp, \
             tc.tile_pool(name="msg", bufs=14) as mp, \
             tc.tile_pool(name="aggps", bufs=3, space="PSUM") as ap, \
             tc.tile_pool(name="p3s", bufs=2) as p3s, \
             tc.tile_pool(name="p3a", bufs=2) as p3a, \
             tc.tile_pool(name="x1p", bufs=2) as x1p, \
             tc.tile_pool(name="pop", bufs=2, space="PSUM") as pop, \
             tc.tile_pool(name="pzp", bufs=1, space="PSUM") as pzp, \
             tc.tile_pool(name="ptp", bufs=2, space="PSUM") as ptp:
            sb_ident = pp.tile([128, 128], f16)
            sb_w = {}
            for k in ["wu1l", "wu1r", "wp1l", "wp1r", "w2A", "w2B"]:
                sb_w[k] = pp.tile([128, 256], f16, tag=k, name=k)
            for k in ["bu1", "bp1"]:
                sb_w[k] = pp.tile([128, 2], f32, tag=k, name=k)
            sb_r8iA = pp.tile([128, SP], f16)
            sb_r8iB = pp.tile([128, SP], f16)

            sb_g1A = gp.tile([128, GMAX], i16, tag="gidx", name="g1A")
            sb_g1B = gp.tile([128, GMAX], i16, tag="gidx", name="g1B")
            nc.sync.dma_start(out=sb_g1A[:, : ntA * 8], in_=t_g["g1A"][:])
            nc.sync.dma_start(out=sb_g1B[:, : ntB * 8], in_=t_g["g1B"][:])
            nc.sync.dma_start(out=sb_ident[:], in_=t_ident[:])
            for k, t in tw.items():
                nc.sync.dma_start(out=sb_w[k][:], in_=t[:])
            nc.sync.dma_start(out=sb_r8iA[:], in_=t_r8iA[:])
            nc.sync.dma_start(out=sb_r8iB[:], in_=t_r8iB[:])

            # zero row of the z tables
            with tc.tile_pool(name="zrow", bufs=1) as zp:
                zt = zp.tile([1, 128], f16)
                nc.vector.memset(zt[:], 0.0)
                nc.sync.dma_start(out=t_zuf[cfg.N: cfg.N + 1, :], in_=zt[:])
                nc.sync.dma_start(out=t_zpf[cfg.N: cfg.N + 1, :], in_=zt[:])

            # ================= aggregation pass emitter =================
            def agg_pass(gidx_sb, T, calls, table_ap, r8i_sb, out_sb,
                         out_parts, lcol, label):
                """Generator: yields after each block's emission."""
                call_of_tile = np.zeros(int(T.sum()), np.int64)
                for k, (t0, ct) in enumerate(calls):
                    call_of_tile[t0: t0 + ct] = k
                msgs = {}

                def chunk_of(tg):
                    k = int(call_of_tile[tg])
                    if k not in msgs:
                        t0, ct = calls[k]
                        m = mp.tile([128, CHUNK_TILES, F_IN], f16,
                                    tag="msg", name=f"msg{label}_{k}")
                        nc.gpsimd.dma_gather(
                            m[:, :ct, :], table_ap,
                            gidx_sb[:, 8 * t0: 8 * t0 + 8 * ct],
                            ct * 128, ct * 128, F_IN)
                        msgs[k] = m
                    return msgs[k], calls[k][0]

                tg = 0
                for b in range(cfg.NB):
                    ps = ap.tile([128, BPD], f32, tag="ps",
                                 name=f"ps{label}_{b}")
                    nt_b = int(T[b])
                    for k in range(nt_b):
                        m, t0 = chunk_of(tg)
                        if lcol is None:
                            lhsT = m[:, tg - t0, :]
                        else:
                            lhsT = m[:, tg - t0,
                                     lcol * CLS: (lcol + 1) * CLS]
                        nc.tensor.matmul(
                            ps[0:out_parts, :], lhsT,
                            r8i_sb[:, b * BPD: (b + 1) * BPD],
                            start=(k == 0), stop=(k == nt_b - 1))
                        tg += 1
                    if lcol is None:
                        nc.vector.tensor_copy(
                            out_sb[0:out_parts, b * BPD: (b + 1) * BPD],
                            ps[0:out_parts, :])
                    else:
                        # layer 2: keep DVE stream clear of these copies (the
                        # scheduler otherwise chains phase-3 stores behind
                        # them); Act engine is nearly idle.
                        nc.scalar.activation(
                            out_sb[0:out_parts, b * BPD: (b + 1) * BPD],
                            ps[0:out_parts, :], AF.Identity)
                    yield b

            # ================= phase-3 emitter (per direction) =================
            def phase3(meanT, xdT_t, wl, wr, b1, w2s, st_z, t_r2, label):
                """Generator: yields after each 512-col group's emission."""
                xdT = bigp.tile([128, SP], f16, tag="big", name=f"xdT{label}")
                nc.sync.dma_start(out=xdT[:], in_=xdT_t[:])
                zrows = p3a.tile([128, cfg.RT, CLS], f16, tag="zrows")
                r2all = p3a.tile([128, SP], f16, tag="r2all")
                ngr = -(-SP // GRP)
                for g in range(ngr):
                    c0 = GRP * g
                    rg = min(GRP, SP - c0)
                    x1g = x1p.tile([128, 2, GRP], f16, tag="x1")
                    po = pop.tile([128, GRP], f32, tag="po")
                    for h in range(2):
                        nc.tensor.matmul(
                            po[:, :rg], wl[:, 128 * h: 128 * (h + 1)],
                            meanT[:, c0: c0 + rg], start=True, stop=False)
                        nc.tensor.matmul(
                            po[:, :rg], wr[:, 128 * h: 128 * (h + 1)],
                            xdT[:, c0: c0 + rg], start=False, stop=True)
                        nc.scalar.activation(
                            x1g[:, h, :rg], po[:, :rg], AF.Relu,
                            bias=b1[:, h: h + 1])
                    pz = pzp.tile([128, GRP], f32, tag="pz")
                    for h in range(2):
                        nc.tensor.matmul(
                            pz[:, :rg], w2s[:, 128 * h: 128 * (h + 1)],
                            x1g[:, h, :rg], start=(h == 0), stop=(h == 1))
                    zr = p3s.tile([128, GRP], f16, tag="zr")
                    nc.vector.tensor_copy(zr[0:CLS, :rg], pz[0:CLS, :rg])
                    # r2 rows (partitions 64:128) accumulate in SBUF
                    nc.vector.tensor_copy(
                        r2all[CLS:128, c0: c0 + rg], pz[CLS:128, :rg])
                    # z rows (partitions 0:64) -> transpose -> zrows
                    for q in range(-(-rg // 128)):
                        cw = min(128, rg - 128 * q)
                        pt = ptp.tile([128, CLS], f16, tag="pt")
                        nc.tensor.transpose(
                            pt[0:cw, :], zr[0:CLS, 128 * q: 128 * q + cw],
                            sb_ident[0:CLS, 0:CLS])
                        nc.vector.tensor_copy(
                            zrows[0:cw, c0 // 128 + q, :], pt[0:cw, :])
                    yield g
                # single batched stores
                nc.sync.dma_start(out=t_r2[:], in_=r2all[CLS:128, :])
                zview = st_z.rearrange("(t p) f -> p t f", p=128)
                nc.sync.dma_start(out=zview[:], in_=zrows[:])

            def drive(agg_gen, p3_gen):
                """Interleave p3 group g right after agg block 8(g+1)-1."""
                done = 0
                for b in agg_gen:
                    while p3_gen is not None and (b + 1) // 8 > done:
                        try:
                            next(p3_gen)
                            done += 1
                        except StopIteration:
                            p3_gen = None
                if p3_gen is not None:
                    for _ in p3_gen:
                        pass

            # ================= emit the whole program =================
            import os as _os
            PARTS = set((_os.environ.get("KERNEL_PARTS") or
                         "agg1,p3,cc,agg2").split(","))

            def cc(st_z, t_zl, label):
                if "cc" not in PARTS:
                    return
                if local_mode:
                    nc.sync.dma_start(out=t_zl[0:S, :], in_=st_z[0:S, :])
                else:
                    nc.gpsimd.collective_compute(
                        "AllGather", mybir.AluOpType.bypass,
                        replica_groups=[list(range(NCORES))],
                        ins=[st_z[0:S, :]], outs=[t_zl[0:cfg.N, :]])

            def expand(t_zl, t_zf):
                # expand tight rows into the 256B-stride gather table
                if "cc" in PARTS:
                    nc.sync.dma_start(
                        out=t_zf[0:cfg.N, 0:CLS], in_=t_zl[0:cfg.N, :])

            meanTA = bigp.tile([128, SP], f16, tag="big", name="meanTA")
            meanTB = bigp.tile([128, SP], f16, tag="big", name="meanTB")
            gA = agg_pass(sb_g1A, TA, c1A, t_xu[CENTER:, :], sb_r8iA,
                          meanTA, 128, None, "A") if "agg1" in PARTS else None
            pA = phase3(meanTA, t_xdTA, sb_w["wu1l"], sb_w["wu1r"],
                        sb_w["bu1"], sb_w["w2A"], st_zu, outs["rp"],
                        "A") if "p3" in PARTS else None
            if gA is not None:
                drive(gA, pA)
            elif pA is not None:
                drive(iter(()), pA)
            tc.no_sync_barrier()
            cc(st_zu, t_zul, "u")

            sb_g2A = gp.tile([128, GMAX], i16, tag="gidx", name="g2A")
            sb_g2B = gp.tile([128, GMAX], i16, tag="gidx", name="g2B")
            nc.sync.dma_start(out=sb_g2A[:, : ntA * 8], in_=t_g["g2A"][:])
            nc.sync.dma_start(out=sb_g2B[:, : ntB * 8], in_=t_g["g2B"][:])

            gB = agg_pass(sb_g1B, TB, c1B, t_xp[CENTER:, :], sb_r8iB,
                          meanTB, 128, None, "B") if "agg1" in PARTS else None
            pB = phase3(meanTB, t_xdTB, sb_w["wp1l"], sb_w["wp1r"],
                        sb_w["bp1"], sb_w["w2B"], st_zp, outs["ru"],
                        "B") if "p3" in PARTS else None
            if gB is not None:
                drive(gB, pB)
            elif pB is not None:
                drive(iter(()), pB)
            tc.no_sync_barrier()
            cc(st_zp, t_zpl, "p")
            expand(t_zul, t_zuf)
            expand(t_zpl, t_zpf)
            tc.no_sync_barrier()

            if "agg2" in PARTS:
                m2A = bigp.tile([128, SP], f16, tag="big", name="m2A")
                for _ in agg_pass(sb_g2A, TA, c2A, t_zuf[CENTER:, :],
                                  sb_r8iA, m2A, CLS, 0, "A2"):
                    pass
                nc.sync.dma_start(out=outs["mu"][:], in_=m2A[0:CLS, :])
                m2B = bigp.tile([128, SP], f16, tag="big", name="m2B")
                for _ in agg_pass(sb_g2B, TB, c2B, t_zpf[CENTER:, :],
                                  sb_r8iB, m2B, CLS, 0, "B2"):
                    pass
                nc.sync.dma_start(out=outs["mp"][:], in_=m2B[0:CLS, :])

    nc.finalize()
    return nc


def build(inputs, cfg=None, local_mode=False):
    cfg = cfg or CFG()
    in_maps, schedA, schedB, host_ctx = _prep_all(inputs, cfg)
    nc = _build_nc(cfg, schedA, schedB, local_mode=local_mode)
    return nc, in_maps, host_ctx


def assemble(results, host_ctx, cfg=None):
    """Un-permute per-core outputs and apply layer-2 biases (host side)."""
    cfg = cfg or CFG()
    S = cfg.S
    xu2 = np.zeros((cfg.N, CLS), np.float32)
    xp2 = np.zeros((cfg.N, CLS), np.float32)
    for c in range(NCORES):
        piA, piB = host_ctx["piA"][c], host_ctx["piB"][c]
        r = results[c]
        xu2[c * S + piA] = r["mu"].astype(np.float32).T[:S]
        xu2[c * S + piB] += r["ru"].astype(np.float32).T[:S]
        xp2[c * S + piB] = r["mp"].astype(np.float32).T[:S]
        xp2[c * S + piA] += r["rp"].astype(np.float32).T[:S]
    xu2 += host_ctx["b_u2"][None, :]
    xp2 += host_ctx["b_p2"][None, :]
    return xu2, xp2


def kernel(**inputs):
    from concourse.bass_utils import run_bass_kernel_spmd

    cfg = CFG()
    nc, in_maps, host_ctx = build(inputs, cfg)
    res = run_bass_kernel_spmd(nc, in_maps, list(range(NCORES)))
    return assemble(res.results, host_ctx, cfg)
